# revision 30
# baseline (speedup 1.0000x reference)
"""Trainium2 Bass kernel for nn_Matcher (gnn_message_passing).

Math: for each of N=4*256*256 graphs with node indices n0..n4 in [0,129):
  sim[g,c] = A[n0,c] + sum_{s=1..4} 0.5*relu(g_s) @ C2_s[:,c]
  g_s = E1[n_s] + sqrt2*E1[n0],  E1 = emb @ W1,  A folds the class branch.

Device kernel (per core, data-parallel over graphs):
- idx rows are DMA-broadcast to 128 partitions as uint8 (halves DMA bytes
  vs bf16); one-hots built on DVE via is_equal against an iota column,
  output directly in fp8e4 (2x_2p DVE mode).
- Stage 1 uses fp8 DoubleRow matmuls: 256-deep contraction pairs
  ktile0=sqrt2*E1 x oh(n0) with ktile1=E1 x oh(n_s) -> one matmul per
  stream at 0.5 cycles/col.
- relu PSUM->fp8 split 7 units on Act, 1 on DVE per super-chunk
  (GPSIMD cannot access PSUM, so only these two can drain stage 1).
- Stage 2 is 3 fp8 DoubleRow matmuls per chunk: stream pairs (1,2) and
  (3,4) share one matmul each (two ktiles = two streams), and the A
  table contracts oh(n0) as a hi+res fp8 pair (A's quantization error
  hits the output directly; C2's averages out over the contraction).
- Output packing: the stage-2 stationary for chunk-position c carries its
  21 columns at offset 32c (zeros elsewhere), so all 4 chunks of a
  super-chunk accumulate into one [128, 512] PSUM tile at different row
  blocks; one engine copy + one strided DMA per 2048 graphs writes HBM.
"""
import numpy as np
import ml_dtypes

N_CORES = 8
B, H, W_DIM = 4, 256, 256
NTOT = B * H * W_DIM            # 262144 graphs
NCORE = NTOT // N_CORES         # 32768
SC = 2048                       # super-chunk (graphs)
CH = 512                        # matmul chunk (graphs); 4 chunks per SC
SQ2 = float(np.sqrt(2.0))
e4 = ml_dtypes.float8_e4m3

_cache = {}


def _build_nc():
    import concourse.bacc as bacc
    import concourse.tile as tile
    import concourse.mybir as mybir

    DR = mybir.MatmulPerfMode.DoubleRow
    IE = mybir.AluOpType.is_equal
    MAX = mybir.AluOpType.max
    RELU = mybir.ActivationFunctionType.Relu

    nc = bacc.Bacc("TRN2", target_bir_lowering=False, debug=False,
                   num_devices=N_CORES)
    idx_d = nc.dram_tensor("idx", [5, NCORE], mybir.dt.uint8,
                           kind="ExternalInput")
    e1p_d = nc.dram_tensor("e1p", [128, 2, 128], mybir.dt.float8e4,
                           kind="ExternalInput")
    vtab_d = nc.dram_tensor("vtab", [4, 3, 128, 2, 128], mybir.dt.float8e4,
                            kind="ExternalInput")
    iota_d = nc.dram_tensor("iota", [128, 1], mybir.dt.float32,
                            kind="ExternalInput")
    # padded output: row 32*b + r holds class r (r<21) of chunk-position b;
    # column sc*CH + c is graph sc*SC + b*CH + c. Host unscrambles.
    out_d = nc.dram_tensor("out", [128, NCORE // 4], mybir.dt.bfloat16,
                           kind="ExternalOutput")

    with tile.TileContext(nc) as tc:
        with (
            tc.tile_pool(name="const", bufs=1) as cpool,
            tc.tile_pool(name="rep", bufs=3) as rpool,
            tc.tile_pool(name="ohp", bufs=3) as opool,
            tc.tile_pool(name="hs", bufs=4) as hpool,
            tc.tile_pool(name="osp", bufs=3) as ospool,
            tc.tile_pool(name="psh", bufs=3, space="PSUM") as phpool,
            tc.tile_pool(name="pso", bufs=2, space="PSUM") as popool,
        ):
            # consts ride the idle gpsimd (SWDGE) queue so the first
            # super-chunk's rep broadcast starts immediately on SP
            iota_t = cpool.tile([128, 1], mybir.dt.float32)
            nc.gpsimd.dma_start(out=iota_t[:], in_=iota_d.ap())
            e1p = cpool.tile([128, 2, 128], mybir.dt.float8e4)
            nc.gpsimd.dma_start(out=e1p[:], in_=e1p_d.ap())
            vtab = cpool.tile([128, 4, 3, 2, 128], mybir.dt.float8e4)
            nc.gpsimd.dma_start(out=vtab[:],
                                in_=vtab_d.ap().transpose([2, 0, 1, 3, 4]))

            for sc in range(NCORE // SC):
                ssl = slice(sc * SC, (sc + 1) * SC)
                rep = rpool.tile([128, 5, SC], mybir.dt.uint8, tag="rep")
                nc.sync.dma_start(
                    out=rep[:, 0:3, :],
                    in_=idx_d.ap()[0:3, ssl].unsqueeze(0)
                    .broadcast_to([128, 3, SC]))
                nc.sync.dma_start(
                    out=rep[:, 3:5, :],
                    in_=idx_d.ap()[3:5, ssl].unsqueeze(0)
                    .broadcast_to([128, 2, SC]))
                oh = opool.tile([128, 5, SC], mybir.dt.float8e4, tag="oh")
                nc.vector.tensor_scalar(out=oh[:, 0:3, :], in0=rep[:, 0:3, :],
                                        scalar1=iota_t[:], scalar2=None,
                                        op0=IE)
                nc.vector.tensor_scalar(out=oh[:, 3:5, :], in0=rep[:, 3:5, :],
                                        scalar1=iota_t[:], scalar2=None,
                                        op0=IE)

                po = popool.tile([128, CH], mybir.dt.float32, tag="po")
                for c in range(SC // CH):
                    csl = slice(c * CH, (c + 1) * CH)
                    # stage 1: 4 DoubleRow matmuls (ktile0=oh0 x sqrt2E1,
                    # ktile1=oh_j x E1), two 2-bank psum tiles
                    phA = phpool.tile([128, 2, CH], mybir.dt.float32,
                                      tag="ph")
                    nc.tensor.matmul(out=phA[:, 0], lhsT=e1p[:],
                                     rhs=oh[:, 0:2:1, csl],
                                     start=True, stop=True, perf_mode=DR)
                    nc.tensor.matmul(out=phA[:, 1], lhsT=e1p[:],
                                     rhs=oh[:, 0:3:2, csl],
                                     start=True, stop=True, perf_mode=DR)
                    phB = phpool.tile([128, 2, CH], mybir.dt.float32,
                                      tag="ph")
                    nc.tensor.matmul(out=phB[:, 0], lhsT=e1p[:],
                                     rhs=oh[:, 0:4:3, csl],
                                     start=True, stop=True, perf_mode=DR)
                    nc.tensor.matmul(out=phB[:, 1], lhsT=e1p[:],
                                     rhs=oh[:, 0:5:4, csl],
                                     start=True, stop=True, perf_mode=DR)
                    # stage 2 A-term first: depends only on oh, so PE
                    # can run it while the relus drain
                    nc.tensor.matmul(
                        out=po[:], lhsT=vtab[:, c, 0],
                        rhs=oh[:, 0:1, csl].broadcast_to([128, 2, CH]),
                        start=(c == 0), stop=False, perf_mode=DR,
                        skip_group_check=True)
                    # relu psum f32 -> fp8; Pool can't read PSUM, so split
                    # 7 units on Act, 1 on DVE per SC
                    hsb = hpool.tile([128, 4, CH], mybir.dt.float8e4,
                                     tag="hsb")
                    nc.scalar.activation(out=hsb[:, 0:2], in_=phA[:],
                                         func=RELU)
                    if c <= 2:
                        nc.scalar.activation(out=hsb[:, 2:4], in_=phB[:],
                                             func=RELU)
                    else:
                        nc.vector.tensor_scalar(out=hsb[:, 2:4], in0=phB[:],
                                                scalar1=0.0, scalar2=None,
                                                op0=MAX)
                    # stream-pair DRs into row block 32c of the shared po
                    nc.tensor.matmul(
                        out=po[:], lhsT=vtab[:, c, 1],
                        rhs=hsb[:, 0:2, :],
                        start=False, stop=False,
                        perf_mode=DR, skip_group_check=True)
                    nc.tensor.matmul(
                        out=po[:], lhsT=vtab[:, c, 2],
                        rhs=hsb[:, 2:4, :],
                        start=False, stop=(c == 3),
                        perf_mode=DR, skip_group_check=True)
                osb = ospool.tile([128, CH], mybir.dt.bfloat16, tag="osb")
                nc.vector.tensor_copy(out=osb[:], in_=po[:])
                nc.gpsimd.dma_start(out=out_d.ap()[:, sc * CH:(sc + 1) * CH],
                                    in_=osb[:])
    nc.compile()
    return nc


def _prepare_consts(class_nodes, emb, W1, b1, W2, b2):
    inv_sqrt2 = np.float32(1.0 / np.sqrt(2.0))
    M = np.zeros((5, 5), dtype=np.float32)
    M[0, 0] = 1.0
    for k in range(1, 5):
        M[k, k] = 0.5
        M[k, 0] = inv_sqrt2

    def gcn(x):
        h = np.einsum('ts,...sd->...td', M, x @ W1) + b1
        h = np.maximum(h, 0)
        return np.einsum('ts,...sd->...td', M, h @ W2) + b2

    OC = gcn(emb[class_nodes]).reshape(21, 5, 21)
    D = np.zeros((21, 5, 21), dtype=np.float32)
    D[:, 0, :] = OC[:, 0, :] + inv_sqrt2 * OC[:, 1:, :].sum(axis=1)
    D[:, 1:, :] = 0.5 * OC[:, 1:, :]
    C2 = np.einsum('kd,ctd->ctk', W2, D)            # [21,5,128]
    K0 = np.einsum('ctd,d->c', OC, b2)              # [21]
    E1 = emb @ W1                                   # [129,128]
    A = np.maximum(E1 + b1, 0) @ C2[:, 0, :].T + K0[None, :]  # [129,21]

    def q(x):
        return x.astype(e4).astype(np.float32)

    e1p = np.zeros((128, 2, 128), dtype=np.float32)
    e1p[:, 0, :] = q(np.float32(SQ2) * E1[:128])
    e1p[:, 1, :] = q(E1[:128])

    # stage-2 tables, 21 columns at offset 32c per chunk-position c:
    # t=0: A as a hi/res fp8 pair (rhs = oh0 on both ktiles);
    # t=1: (T1, T2) stream pair; t=2: (T3, T4) -- plain fp8 each
    # (C2 quantization error averages out over the 128-contraction; A's
    # does not, hence hi/res for A only).
    Ts = [0.5 * C2[:, s, :].T for s in range(1, 5)]  # [128,21] each
    Ahi = q(A[:128])
    vtab = np.zeros((4, 3, 128, 2, 128), dtype=np.float32)
    for c in range(4):
        o = 32 * c
        vtab[c, 0, :, 0, o:o + 21] = Ahi
        vtab[c, 0, :, 1, o:o + 21] = q(A[:128] - Ahi)
        vtab[c, 1, :, 0, o:o + 21] = q(Ts[0])
        vtab[c, 1, :, 1, o:o + 21] = q(Ts[1])
        vtab[c, 2, :, 0, o:o + 21] = q(Ts[2])
        vtab[c, 2, :, 1, o:o + 21] = q(Ts[3])

    return {
        "e1p": e1p.astype(e4),
        "vtab": vtab.astype(e4),
        "iota": np.arange(128, dtype=np.float32)[:, None],
    }


def _make_in_maps(instance_nodes, class_nodes, emb, W1, b1, W2, b2):
    consts = _prepare_consts(class_nodes, emb, W1, b1, W2, b2)
    n = np.asarray(instance_nodes).reshape(NTOT, 5).astype(np.int32)
    idx_u8 = np.ascontiguousarray(n.T).astype(np.uint8)   # [5, NTOT]
    in_maps = []
    for i in range(N_CORES):
        m = dict(consts)
        m["idx"] = np.ascontiguousarray(idx_u8[:, i * NCORE:(i + 1) * NCORE])
        in_maps.append(m)
    return in_maps


def kernel(instance_nodes, class_nodes, emb, W1, b1, W2, b2):
    instance_nodes = np.asarray(instance_nodes)
    class_nodes = np.asarray(class_nodes).astype(np.int64)
    emb = np.asarray(emb, dtype=np.float32)
    W1 = np.asarray(W1, dtype=np.float32)
    b1 = np.asarray(b1, dtype=np.float32)
    W2 = np.asarray(W2, dtype=np.float32)
    b2 = np.asarray(b2, dtype=np.float32)

    if "nc" not in _cache:
        _cache["nc"] = _build_nc()
    nc = _cache["nc"]

    in_maps = _make_in_maps(instance_nodes, class_nodes, emb, W1, b1, W2, b2)

    from concourse.bass_utils import run_bass_kernel_spmd
    res = run_bass_kernel_spmd(nc, in_maps, list(range(N_CORES)))
    outs = []
    for i in range(N_CORES):
        arr = np.asarray(res.results[i]["out"], dtype=np.float32)
        a4 = arr.reshape(4, 32, NCORE // SC, CH)    # [b, r, sc, c]
        outs.append(np.transpose(a4[:, :21], (1, 2, 0, 3)).reshape(21, NCORE))
    out = np.concatenate(outs, axis=1)              # [21, NTOT]
    sim = np.ascontiguousarray(out.T).reshape(B, H, W_DIM, 21)
    return sim.astype(np.float32)
